# revision 54
# baseline (speedup 1.0000x reference)
"""Trainium2 Bass kernel for a Swin-style transformer block.

Reference computation (per image, H=W=64, C=384, 12 heads, 8x8 windows):
  x -> LN1 -> qkv -> windowed MHA (+rel-pos bias) -> proj -> +x
    -> LN2 -> fc1 -> ReLU6 -> fc2 -> +residual

Sharding: data-parallel over batch (16 images -> 8 cores x 2 images).

Per-core kernel design notes:
 - Tokens are processed window-major: tiles of 128 tokens = one "window pair"
   (two 8x8 windows); 4 window pairs = one 512-token chunk; 16 chunks/core.
 - LayerNorm gamma/beta are folded into the following matmul's weights and
   bias host-side (exact), so the normalized tile (x-mu)*rstd is transposed
   straight into matmul layout with SBUF-SBUF DMA transposes (no PE identity
   matmuls, no PSUM evacuation traffic for the transposes).
 - q/k are head-split with cheap HWDGE DMAs (SP + Act queues) into
   [32, 12, tok] tiles; V is computed full-width (both windows of the pair
   on 128 partitions) with a small shift DMA for half B.  All matmul
   operands and outputs sit at partition base 0 (quadrant tile_position
   matmuls are unreliable on this device stack).  Logits fill single-bank
   PSUM tiles 8 (half, head)-slots at a time so exp of bank b overlaps QK
   fills of bank b+1; AV/proj trail one window pair behind QK so the
   exp->exb chain latency hides behind PE work.  V is augmented with a
   ones column and attnT (exponentiated, bias-folded) is the stationary
   operand of attnT.T @ [V|1]; the output holds both the unnormalized
   attention output and the softmax denominator, normalized with one
   reciprocal + multiply.  No max-subtraction (logits are bounded for this
   distribution).  The DMA transposes back to token-major also lift half B
   to partitions 64-127 for free.
 - 5-stage staggered software pipeline (load+LN1 / qkv+attention+proj+LN2 /
   fp8 cast / fc1+clamp / fc2+store) so every DMA-transpose result is
   consumed a full round after issue (in-order engine queues head-of-line
   block otherwise).
 - The relative-position bias is folded in as a precomputed exp(bias)
   elementwise multiply on gpsimd (exp(l+b) = exp(l)*exp(b)).
 - MLP stays feature-major end to end; ReLU6 splits between DVE (fused
   clamp evac) and Act+gpsimd by feature-chunk parity.  KERNEL_PREC=
   bf16-mlp8 runs fc1/fc2 in fp8e4 DoubleRow (2x fewer PE cycles) but its
   ~3e-2 rel err exceeds the 2e-2 gate, so bf16 is the default.
"""

import os
import numpy as np

# ---------------------------------------------------------------- constants
B, L, C = 16, 4096, 384
HEADS, WS, HD = 12, 8, 32
MLP = 1536
NCORES = 8
BPC = B // NCORES          # images per core
T = BPC * L                # tokens per core
H = W = 64
EPS = 1e-5
NWIN = BPC * (H // WS) * (W // WS)   # 128 windows/core
NWP = NWIN // 2                      # 64 window pairs
WP_PER_CHUNK = int(os.environ.get("KERNEL_WP", "4"))   # window pairs per chunk
TOK = WP_PER_CHUNK * 128             # tokens per chunk
NCHUNK = NWP // WP_PER_CHUNK         # 16

DEFAULT_PREC = os.environ.get("KERNEL_PREC", "bf16")

_BUILD_CACHE = {}


def _rel_pos_index():
    coords = np.stack(np.meshgrid(np.arange(WS), np.arange(WS), indexing="ij"))
    cf = coords.reshape(2, -1)
    rel = cf[:, :, None] - cf[:, None, :]
    rel = rel.transpose(1, 2, 0).astype(np.int64)
    rel[:, :, 0] += WS - 1
    rel[:, :, 1] += WS - 1
    rel[:, :, 0] *= 2 * WS - 1
    return rel.sum(-1)  # (64, 64)


def _split_excess_waits(nc, max_waits=1):
    """TRN2 instructions encode a single semaphore-wait slot; Tile's exit
    drain (and occasionally other instructions) carries several.  Hoist the
    excess into standalone event-semaphore waits on the same engine."""
    import concourse.mybir as mybir

    uid = [0]
    for fn in nc.m.functions:
        for bb in fn.blocks:
            out = []
            for ins in bb.instructions:
                si = ins.sync_info
                if si is not None and si.on_wait and len(si.on_wait) > max_waits:
                    waits = list(si.on_wait)
                    excess, keep = waits[:-max_waits], waits[-max_waits:]
                    for w in excess:
                        uid[0] += 1
                        ev = mybir.InstEventSemaphore(
                            name=f"WSPLIT-{uid[0]}",
                            engine=ins.engine,
                            ins=[],
                            outs=[],
                            sync_info=mybir.SyncInfo(on_wait=[w], on_update=[]),
                        )
                        nc.register_instruction(ev, overwrite=True)
                        out.append(ev)
                    si.on_wait = keep
                out.append(ins)
            bb.instructions = out


def _build(prec, has_fc1b, has_projb, has_fc2b, has_vb=False, stage="full"):
    import concourse.bass as bass
    import concourse.mybir as mybir
    from concourse.tile import TileContext

    f32 = mybir.dt.float32
    bf16 = mybir.dt.bfloat16
    fp8 = mybir.dt.float8e4
    mlp8 = prec.endswith("-mlp8")
    base_prec = prec.replace("-mlp8", "")
    assert base_prec == "bf16", prec
    DT_D = DT_A = bf16
    DT_M = fp8 if mlp8 else DT_D          # MLP operand dtype

    nc = bass.Bass()

    x_d = nc.declare_dram_parameter("x", [NWP, 128, C], f32, isOutput=False)
    o_d = nc.declare_dram_parameter("o", [NWP, 128, C], f32, isOutput=True)
    wqkvT_d = nc.declare_dram_parameter("wqkvT", [C, 3 * C], DT_D, isOutput=False)
    wpT_d = nc.declare_dram_parameter("wpT", [C, C], DT_D, isOutput=False)
    w1T_d = nc.declare_dram_parameter(
        "w1T", [512 if mlp8 else C, MLP], DT_M, isOutput=False
    )
    w2T_d = nc.declare_dram_parameter("w2T", [MLP, C], DT_M, isOutput=False)
    qkb_d = nc.declare_dram_parameter("qkb", [C, 2], f32, isOutput=False)
    vb_d = nc.declare_dram_parameter("vb", [C], f32, isOutput=False)
    fc1b_d = nc.declare_dram_parameter("fc1b", [MLP], f32, isOutput=False)
    cb_d = nc.declare_dram_parameter("cb", [C, 2], f32, isOutput=False)  # proj_b, fc2_b
    expb_d = nc.declare_dram_parameter("expb", [64, 2 * HEADS, 64], DT_A, isOutput=False)

    AL = mybir.AluOpType
    AF = mybir.ActivationFunctionType
    DR = mybir.MatmulPerfMode.DoubleRow

    from contextlib import ExitStack

    with TileContext(nc) as tc, ExitStack() as _stk:
            pool = lambda name, bufs, **kw: _stk.enter_context(
                tc.tile_pool(name=name, bufs=bufs, **kw)
            )
            consts = pool("consts", 1)
            px = pool("px", int(os.environ.get("KB_X", "3")))
            pt = pool("pt", int(os.environ.get("KB_T", "2")))
            pstat = pool("pstat", int(os.environ.get("KB_STAT", "2")))
            pxlnT = pool("pxlnT", int(os.environ.get("KB_XLNT", "2")))
            pqkT = pool("pqkT", 2)
            pqh = pool("pqh", int(os.environ.get("KB_QH", "1")))
            pV = pool("pV", int(os.environ.get("KB_V", "1")))
            pexp = pool("pexp", int(os.environ.get("KB_EXP", "2")))
            po = pool("po", int(os.environ.get("KB_O", "2")))
            poT = pool("poT", 2)
            px2 = pool("px2", int(os.environ.get("KB_X2", "4")))
            ph2T = pool("ph2T", 2)
            ph3 = pool("ph3", int(os.environ.get("KB_H3", "2")))
            pout = pool("pout", 2)
            _pb = [int(v) for v in os.environ.get("KERNEL_PSUM", "1,3,2,2").split(",")]
            psMM = pool("psMM", _pb[0], space="PSUM")
            psQK = pool("psQK", _pb[1], space="PSUM")
            psAV = pool("psAV", _pb[2], space="PSUM")
            psF = pool("psF", _pb[3], space="PSUM")
            # ---------------- constants into SBUF
            wqkvT = consts.tile([128, 3, 3 * C], DT_D, tag="wqkvT")
            nc.sync.dma_start(
                out=wqkvT, in_=wqkvT_d[:].rearrange("(a p) o -> p a o", p=128)
            )
            wpT = consts.tile([128, 3, C], DT_D, tag="wpT")
            nc.sync.dma_start(out=wpT, in_=wpT_d[:].rearrange("(a p) o -> p a o", p=128))
            w1T = consts.tile([128, 4 if mlp8 else 3, MLP], DT_M, tag="w1T")
            nc.sync.dma_start(out=w1T, in_=w1T_d[:].rearrange("(a p) o -> p a o", p=128))
            w2T = consts.tile([128, 12, C], DT_M, tag="w2T")
            nc.sync.dma_start(out=w2T, in_=w2T_d[:].rearrange("(a p) o -> p a o", p=128))
            qkb = consts.tile([128, 3, 2], f32, tag="qkb")
            nc.sync.dma_start(out=qkb, in_=qkb_d[:].rearrange("(a p) s -> p a s", p=128))
            expb = consts.tile([64, 2 * HEADS, 64], DT_A, tag="expb")
            nc.sync.dma_start(out=expb, in_=expb_d[:])
            vb = consts.tile([128, C], f32, tag="vb")
            nc.gpsimd.dma_start(out=vb, in_=vb_d[:].partition_broadcast(128))
            epst = consts.tile([128, 1], f32, tag="eps")
            nc.vector.memset(epst[:], EPS)
            fc1b = None
            if has_fc1b:
                fc1b = consts.tile([128, 12], f32, tag="fc1b")
                nc.sync.dma_start(
                    out=fc1b, in_=fc1b_d[:].rearrange("(a p) -> p a", p=128)
                )
            cbias = None
            if has_projb or has_fc2b:
                cbias = consts.tile([128, C, 2], f32, tag="cb")
                nc.gpsimd.dma_start(
                    out=cbias, in_=cb_d[:].partition_broadcast(128)
                )
            zero8 = None
            if mlp8:
                zero8 = consts.tile([128, TOK], fp8, tag="zero8")
                nc.vector.memset(zero8[:], 0.0)

            tt_eng = os.environ.get("KERNEL_TT", "dve")

            def ln_stage_j(src_tile, dst_T_tiles, j):
                """token-major LN for one window pair: src [128,384] f32 ->
                dst_T [128, 128j:128j+128] bf16 (transposed; gamma/beta
                pre-folded into the next matmul)."""
                if True:
                    st = pstat.tile([128, 6], f32, tag=f"bn{j}")
                    nc.vector.bn_stats(out=st, in_=src_tile[:])
                    mv = pstat.tile([128, 2], f32, tag=f"mv{j}")
                    nc.vector.bn_aggr(out=mv, in_=st)
                    # rstd = exp(-0.5*ln(var+eps)): keeps all ACT funcs in the
                    # natural_log_exp table set (one table load for the kernel)
                    rst = pstat.tile([128, 2], f32, tag=f"rs{j}")
                    nc.scalar.activation(
                        out=rst[:, 0:1], in_=mv[:, 1:2], func=AF.Ln,
                        bias=epst[:, 0:1], scale=1.0,
                    )
                    nc.scalar.activation(
                        out=rst[:, 1:2], in_=rst[:, 0:1], func=AF.Exp, bias=0.0, scale=-0.5
                    )
                    tt = pt.tile([128, C], DT_D, tag=f"t{j}")
                    if tt_eng == "act":
                        nmr = pstat.tile([128, 1], f32, tag=f"nm{j}")
                        nc.vector.tensor_scalar(
                            out=nmr[:], in0=mv[:, 0:1], scalar1=rst[:, 1:2],
                            scalar2=-1.0, op0=AL.mult, op1=AL.mult,
                        )
                        nc.scalar.activation(
                            out=tt[:], in_=src_tile[:], func=AF.Identity,
                            bias=nmr[:, 0:1], scale=rst[:, 1:2],
                        )
                    else:
                        nc.vector.tensor_scalar(
                            out=tt[:],
                            in0=src_tile[:],
                            scalar1=mv[:, 0:1],
                            scalar2=rst[:, 1:2],
                            op0=AL.subtract,
                            op1=AL.mult,
                        )
                    for cc in range(3):
                        nc.sync.dma_start_transpose(
                            dst_T_tiles[cc][:, 128 * j : 128 * (j + 1)],
                            tt[:, 128 * cc : 128 * (cc + 1)],
                        )

            def ln_stage(src_tiles, dst_T_tiles):
                for j in range(WP_PER_CHUNK):
                    ln_stage_j(src_tiles[j], dst_T_tiles, j)

            # ====== 5-stage software pipeline, staggered so every
            # DMA-transpose result is consumed a full round after issue
            # (in-order engine queues head-of-line block otherwise):
            #   round r emits A(r), B(r-1), C(r-1), D(r-2), E(r-3)
            def stageA(ci):
                wp0 = ci * WP_PER_CHUNK
                x_tm = []
                for j in range(WP_PER_CHUNK):
                    xt = px.tile([128, C], f32, tag=f"x{j}")
                    nc.sync.dma_start(out=xt[:], in_=x_d[wp0 + j])
                    x_tm.append(xt)
                xlnT = [pxlnT.tile([128, TOK], DT_D, tag=f"xlnT{cc}", name=f"xlnT{cc}") for cc in range(3)]
                ln_stage(x_tm, xlnT)
                return {"wp0": wp0, "x_tm": x_tm, "xlnT": xlnT}

            def passthrough(st):
                for tt in range(WP_PER_CHUNK):
                    out_t = pout.tile([128, C], f32, tag=f"out{tt}")
                    nc.vector.tensor_copy(out=out_t[:], in_=st["x_tm"][tt][:])
                    nc.sync.dma_start(out=o_d[st["wp0"] + tt], in_=out_t[:])

            def stageB(st):
                x_tm, xlnT = st["x_tm"], st["xlnT"]
                # ---- qkv -> qT/kT [128,512] per output chunk (4 heads
                # each), then head-split into [32, 12, 512] via strided
                # HWDGE DMAs (quadrant matmuls are unreliable on device;
                # operands must sit at partition base 0)
                qT, kT = [], []
                for oc in range(3):
                    for which, dst_list, bcol in (("q", qT, 0), ("k", kT, 1)):
                        ps = psMM.tile([128, 512], f32, tag="mm")
                        for kc in range(3):
                            col0 = (0 if which == "q" else C) + 128 * oc
                            nc.tensor.matmul(
                                ps[:, 0:TOK],
                                lhsT=wqkvT[:, kc, col0 : col0 + 128],
                                rhs=xlnT[kc][:],
                                start=(kc == 0),
                                stop=(kc == 2),
                            )
                        dst = pqkT.tile([128, TOK], DT_A, tag=f"{which}T{oc}")
                        # whole-tile evac (the head-split gather DMA below
                        # reads a partition-rearranged view; split writes
                        # break its dependency tracking): q on Act, k on DVE
                        if which == "q":
                            nc.scalar.activation(
                                out=dst[:], in_=ps[:, 0:TOK], func=AF.Identity,
                                bias=qkb[:, oc, bcol : bcol + 1], scale=1.0,
                            )
                        else:
                            nc.vector.tensor_scalar(
                                out=dst[:], in0=ps[:, 0:TOK],
                                scalar1=qkb[:, oc, bcol : bcol + 1], scalar2=None,
                                op0=AL.add,
                            )
                        dst_list.append(dst)
                qh = pqh.tile([32, HEADS, TOK], DT_A, tag="qh", name="qh")
                kh = pqh.tile([32, HEADS, TOK], DT_A, tag="kh", name="kh")
                # plain partition-slice reads (a partition-rearranged view
                # in one DMA defeats the dependency tracker)
                for oc in range(3):
                    for hh in range(4):
                        nc.sync.dma_start(
                            out=qh[:, 4 * oc + hh, :],
                            in_=qT[oc][32 * hh : 32 * hh + 32, :],
                        )
                        nc.scalar.dma_start(
                            out=kh[:, 4 * oc + hh, :],
                            in_=kT[oc][32 * hh : 32 * hh + 32, :],
                        )

                # ---- V (full window pair on 128 partitions, ones column);
                # half B shifted to partition base 0 with a small DMA
                V_aug, V_augB = [], []
                for j in range(WP_PER_CHUNK):
                    ps = psMM.tile([128, 512], f32, tag="mm")
                    for kc in range(3):
                        nc.tensor.matmul(
                            ps[:, :C],
                            lhsT=xlnT[kc][:, 128 * j : 128 * (j + 1)],
                            rhs=wqkvT[:, kc, 2 * C : 3 * C],
                            start=(kc == 0),
                            stop=(kc == 2),
                        )
                    va = pV.tile([128, HEADS, HD + 1], DT_A, tag=f"va{j}", name=f"va{j}")
                    if not has_vb and os.environ.get("KERNEL_VAE", "dve") == "act":
                        nc.scalar.activation(
                            out=va[:, 0:6, 0:HD],
                            in_=ps[:, 0 : C // 2].rearrange("p (h d) -> p h d", h=HEADS // 2),
                            func=AF.Identity, bias=0.0, scale=1.0,
                        )
                        nc.vector.tensor_scalar(
                            out=va[:, 6:12, 0:HD],
                            in0=ps[:, C // 2 : C].rearrange("p (h d) -> p h d", h=HEADS // 2),
                            scalar1=0.0, scalar2=None, op0=AL.add,
                        )
                    else:
                        nc.vector.scalar_tensor_tensor(
                            out=va[:, :, 0:HD],
                            in0=ps[:, :C].rearrange("p (h d) -> p h d", h=HEADS),
                            scalar=0.0,
                            in1=vb[:].rearrange("p (h d) -> p h d", h=HEADS),
                            op0=AL.add,
                            op1=AL.add,
                        )
                    nc.vector.memset(va[:, :, HD : HD + 1], 1.0)
                    vab = pV.tile([64, HEADS, HD + 1], DT_A, tag=f"vab{j}", name=f"vab{j}")
                    nc.sync.dma_start(out=vab[:], in_=va[64:128])
                    V_aug.append(va)
                    V_augB.append(vab)

                if stage == "qkv":
                    passthrough(st)
                    return False

                # ---- attention; AV/proj run one window pair behind QK so
                # the exp->exb chain latency is hidden behind PE work
                oT = [poT.tile([128, TOK], DT_A, tag=f"oT{cc}", name=f"oT{cc}") for cc in range(3)]
                x2_tm = [None] * WP_PER_CHUNK
                exb_eng = os.environ.get("KERNEL_EXB", "pool")

                def consume(j, exb):
                    for half in (0, 1):
                        vsrc = V_aug[j][0:64] if half == 0 else V_augB[j]
                        psav = psAV.tile([64, HEADS, HD + 2], f32, tag="av", name="psav")
                        for h in range(HEADS):
                            nc.tensor.matmul(
                                psav[:, h, 0 : HD + 1],
                                lhsT=exb[:, HEADS * half + h, :],
                                rhs=vsrc[:, h, 0 : HD + 1],
                                start=True,
                                stop=True,
                                skip_group_check=True,
                            )
                        rec = pstat.tile([64, HEADS], f32, tag="rec")
                        nc.vector.reciprocal(out=rec[:], in_=psav[:, :, HD : HD + 1])
                        ow = po.tile(
                            [64, C], DT_A, tag=f"o{2 * j + half}", name=f"o{2 * j + half}"
                        )
                        nc.vector.tensor_tensor(
                            out=ow[:].rearrange("p (h d) -> p h d", h=HEADS),
                            in0=psav[:, :, 0:HD],
                            in1=rec[:, :, None].broadcast_to([64, HEADS, HD]),
                            op=AL.mult,
                        )
                        # transpose (also lifts half B to token partitions
                        # 64-127)
                        for cc in range(3):
                            nc.sync.dma_start_transpose(
                                oT[cc][:, 128 * j + 64 * half : 128 * j + 64 * half + 64],
                                ow[:, 128 * cc : 128 * (cc + 1)],
                            )

                h2T = [ph2T.tile([128, TOK], DT_D, tag=f"h2T{cc}", name=f"h2T{cc}") for cc in range(3)]

                def proj(tt):
                    ps = psMM.tile([128, 512], f32, tag="mm")
                    for cc in range(3):
                        nc.tensor.matmul(
                            ps[:, :C],
                            lhsT=oT[cc][:, 128 * tt : 128 * (tt + 1)],
                            rhs=wpT[:, cc, :],
                            start=(cc == 0),
                            stop=(cc == 2),
                        )
                    x2 = px2.tile([128, C], f32, tag=f"x2_{tt}")
                    nc.vector.scalar_tensor_tensor(
                        out=x2[:], in0=ps[:, :C], scalar=0.0, in1=x_tm[tt][:],
                        op0=AL.add, op1=AL.add,
                    )
                    if has_projb:
                        nc.vector.tensor_add(x2[:], x2[:], cbias[:, :, 0])
                    x2_tm[tt] = x2
                    # LN2 for this window pair immediately: its transposes
                    # issue a full round before fc1 consumes h2T
                    ln_stage_j(x2, h2T, tt)

                pend_av = []
                for j in range(WP_PER_CHUNK):
                    ja = 128 * j
                    # 24 (half, head) logit slots, 8 per single-bank PSUM
                    # tile so exp of bank b overlaps QK fills of bank b+1
                    psq = [
                        psQK.tile([64, 8, 64], f32, tag="qk", name="psq")
                        for _ in range(3)
                    ]
                    ex = pexp.tile([64, 2 * HEADS, 64], DT_A, tag="ex")
                    exb = pexp.tile([64, 2 * HEADS, 64], DT_A, tag="exb")
                    for b in range(3):
                        for s in range(8):
                            slot = 8 * b + s
                            half, h = slot // HEADS, slot % HEADS
                            t0 = ja + 64 * half
                            nc.tensor.matmul(
                                psq[b][:, s, :],
                                lhsT=kh[:, h, t0 : t0 + 64],
                                rhs=qh[:, h, t0 : t0 + 64],
                                start=True,
                                stop=True,
                                skip_group_check=True,
                            )
                        if stage == "qkm":
                            continue
                        nc.scalar.activation(
                            out=ex[:, 8 * b : 8 * b + 8, :],
                            in_=psq[b][:],
                            func=AF.Exp,
                        )
                        if exb_eng == "pool":
                            nc.gpsimd.tensor_tensor(
                                out=exb[:, 8 * b : 8 * b + 8, :],
                                in0=ex[:, 8 * b : 8 * b + 8, :],
                                in1=expb[:, 8 * b : 8 * b + 8, :],
                                op=AL.mult,
                            )
                        else:
                            nc.vector.tensor_mul(
                                exb[:, 8 * b : 8 * b + 8, :],
                                ex[:, 8 * b : 8 * b + 8, :],
                                expb[:, 8 * b : 8 * b + 8, :],
                            )
                    if stage in ("qk", "qkm"):
                        continue
                    pend_av.append((j, exb))
                    if len(pend_av) > 1:
                        consume(*pend_av.pop(0))
                        proj(j - 1)
                if stage in ("att", "qk", "qkm"):
                    while pend_av:
                        consume(*pend_av.pop(0))
                    passthrough(st)
                    return False
                while pend_av:
                    consume(*pend_av.pop(0))
                    proj(WP_PER_CHUNK - 1)
                st["x2_tm"] = x2_tm
                st["h2T"] = h2T
                return True

            def stageC(st):
                h2T = st["h2T"]
                if mlp8:
                    # single [128, 4, 512] fp8 tile: 3 cast chunks + 1 zero
                    # chunk so fc1 runs as two clean DoubleRow matmuls
                    h2T8 = ph2T.tile([128, 4, TOK], fp8, tag="h2T8", name="h2T8")
                    for cc in range(3):
                        nc.gpsimd.dma_start(out=h2T8[:, cc, :], in_=h2T[cc][:])
                    nc.gpsimd.dma_start(out=h2T8[:, 3, :], in_=zero8[:])
                    st["h2T8"] = h2T8
                st["h2T"] = h2T

            def stageD(st):
                # ---- fc1 + relu6 (feature-major); h3 stored as 6
                # pair-tiles [128, 2, 512] so fc2 can consume DoubleRow
                # pairs directly
                h2T = st["h2T"]
                h3 = [
                    ph3.tile([128, 2, TOK], DT_M, tag=f"h3_{p}", name=f"h3_{p}")
                    for p in range(6)
                ]
                for mc in range(12):
                    ps = psF.tile([128, 512], f32, tag="mm2")
                    if mlp8:
                        h2T8 = st["h2T8"]
                        for p in range(2):
                            nc.tensor.matmul(
                                ps[:, 0:TOK],
                                lhsT=w1T[:, 2 * p : 2 * p + 2, 128 * mc : 128 * (mc + 1)],
                                rhs=h2T8[:, 2 * p : 2 * p + 2, :],
                                start=(p == 0), stop=(p == 1),
                                perf_mode=DR,
                            )
                    else:
                        for kc in range(3):
                            nc.tensor.matmul(
                                ps[:, 0:TOK],
                                lhsT=w1T[:, kc, 128 * mc : 128 * (mc + 1)],
                                rhs=h2T[kc][:],
                                start=(kc == 0),
                                stop=(kc == 2),
                            )
                    h3t = h3[mc // 2][:, mc % 2, :]
                    h3_mode = os.environ.get("KERNEL_H3", "split")
                    if h3_mode == "split":
                        h3_mode = "actpool" if mc % 2 else "dve"
                    if h3_mode == "actpool":
                        # relu evac on Act (fc1 bias is per-partition here:
                        # features on partitions), min(6) on gpsimd
                        nc.scalar.activation(
                            out=h3t, in_=ps[:, 0:TOK], func=AF.Relu,
                            bias=(fc1b[:, mc : mc + 1] if has_fc1b else 0.0),
                            scale=1.0,
                        )
                        nc.gpsimd.tensor_scalar(
                            out=h3t, in0=h3t, scalar1=6.0, scalar2=None,
                            op0=AL.min,
                        )
                    elif has_fc1b:
                        nc.vector.tensor_scalar(
                            out=h3t, in0=ps[:, 0:TOK],
                            scalar1=fc1b[:, mc : mc + 1], scalar2=0.0,
                            op0=AL.add, op1=AL.max,
                        )
                        nc.gpsimd.tensor_scalar(
                            out=h3t, in0=h3t, scalar1=6.0, scalar2=None,
                            op0=AL.min,
                        )
                    else:
                        nc.vector.tensor_scalar(
                            out=h3t, in0=ps[:, 0:TOK], scalar1=0.0, scalar2=6.0,
                            op0=AL.max, op1=AL.min,
                        )
                st["h3"] = h3

            def stageE(st):
                wp0, x2_tm, h3 = st["wp0"], st["x2_tm"], st["h3"]
                for tt in range(WP_PER_CHUNK):
                    ps = psF.tile([128, 512], f32, tag="mm2")
                    if mlp8:
                        for p in range(6):
                            nc.tensor.matmul(
                                ps[:, :C],
                                lhsT=h3[p][:, :, 128 * tt : 128 * (tt + 1)],
                                rhs=w2T[:, 2 * p : 2 * p + 2, :],
                                start=(p == 0), stop=(p == 5),
                                perf_mode=DR,
                            )
                    else:
                        for p in range(6):
                            for i in range(2):
                                mc = 2 * p + i
                                nc.tensor.matmul(
                                    ps[:, :C],
                                    lhsT=h3[p][:, i, 128 * tt : 128 * (tt + 1)],
                                    rhs=w2T[:, mc, :],
                                    start=(mc == 0),
                                    stop=(mc == 11),
                                )
                    out_t = pout.tile([128, C], f32, tag=f"out{tt}")
                    nc.vector.scalar_tensor_tensor(
                        out=out_t[:], in0=ps[:, :C], scalar=0.0, in1=x2_tm[tt][:],
                        op0=AL.add, op1=AL.add,
                    )
                    if has_fc2b:
                        nc.vector.tensor_add(out_t[:], out_t[:], cbias[:, :, 1])
                    nc.sync.dma_start(out=o_d[wp0 + tt], in_=out_t[:])

            sts = {}
            for r in range(NCHUNK + 3):
                if r < NCHUNK:
                    sts[r] = stageA(r)
                    if stage == "ln":
                        passthrough(sts[r])
                        sts.pop(r)
                        continue
                if stage == "ln":
                    continue
                if r - 1 in sts and "x2_tm" not in sts[r - 1]:
                    if not stageB(sts[r - 1]):
                        sts.pop(r - 1)
                        continue
                    stageC(sts[r - 1])
                if r - 2 in sts:
                    stageD(sts[r - 2])
                if r - 3 in sts:
                    stageE(sts[r - 3])
                    sts.pop(r - 3)

    _split_excess_waits(nc, 1)
    return nc


def _prep_inputs(inputs, prec):
    import ml_dtypes

    bf16 = ml_dtypes.bfloat16
    fp8 = ml_dtypes.float8_e4m3
    mlp8 = prec.endswith("-mlp8")
    dt_d = bf16
    dt_a = bf16
    dt_m = fp8 if mlp8 else dt_d

    f = lambda a: np.ascontiguousarray(np.asarray(a, dtype=np.float32))
    x = f(inputs["x"])
    qkv_w, qkv_b = f(inputs["qkv_w"]), f(inputs["qkv_b"])
    ln1_g, ln1_b = f(inputs["ln1_g"]), f(inputs["ln1_b"])
    ln2_g, ln2_b = f(inputs["ln2_g"]), f(inputs["ln2_b"])
    # fold LN1 gamma/beta into qkv (exact):  W @ (g*z + b) = (W*g) @ z + W @ b
    qkv_w_f = qkv_w * ln1_g[None, :]
    qkv_b_f = qkv_b + qkv_w @ ln1_b
    scale = 1.0 / np.sqrt(HD)
    wq = qkv_w_f[0:C] * scale
    wqkvT = np.concatenate([wq.T, qkv_w_f[C : 2 * C].T, qkv_w_f[2 * C :].T], axis=1)
    qkb = np.stack([qkv_b_f[0:C] * scale, qkv_b_f[C : 2 * C]], axis=1)
    vb = qkv_b_f[2 * C :]
    wpT = f(inputs["proj_w"]).T
    # fold LN2 gamma/beta into fc1 (exact)
    fc1_w, fc1_b = f(inputs["fc1_w"]), f(inputs["fc1_b"])
    fc1_w_f = fc1_w * ln2_g[None, :]
    fc1_b_f = fc1_b + fc1_w @ ln2_b
    w1T = fc1_w_f.T
    if mlp8:
        w1T = np.concatenate([w1T, np.zeros((512 - C, MLP), np.float32)], axis=0)
    w2T = f(inputs["fc2_w"]).T
    cb = np.stack([f(inputs["proj_b"]), f(inputs["fc2_b"])], axis=1)

    rel = _rel_pos_index()
    bias = f(inputs["rpb_table"])[rel]          # [n, m, HEADS]
    expb1 = np.exp(bias.transpose(1, 2, 0))     # [m, HEADS, n]
    expb = np.concatenate([expb1, expb1], axis=1)  # [m, 2*HEADS, n] half-major

    common = {
        "wqkvT": np.ascontiguousarray(wqkvT.astype(dt_d)),
        "wpT": np.ascontiguousarray(wpT.astype(dt_d)),
        "w1T": np.ascontiguousarray(w1T.astype(dt_m)),
        "w2T": np.ascontiguousarray(w2T.astype(dt_m)),
        "qkb": np.ascontiguousarray(qkb),
        "vb": np.ascontiguousarray(vb),
        "fc1b": np.ascontiguousarray(fc1_b_f),
        "cb": np.ascontiguousarray(cb),
        "expb": np.ascontiguousarray(expb.astype(dt_a)),
    }
    flags = (
        bool(np.any(fc1_b_f)),
        bool(np.any(cb[:, 0])),
        bool(np.any(cb[:, 1])),
        bool(np.any(vb)),
    )
    in_maps = []
    for c in range(NCORES):
        m = dict(common)
        xc = x[c * BPC : (c + 1) * BPC].reshape(BPC, 8, 8, 4, 2, 8, C)
        m["x"] = np.ascontiguousarray(
            xc.transpose(0, 1, 3, 4, 2, 5, 6).reshape(NWP, 128, C)
        )
        in_maps.append(m)
    return in_maps, flags


def kernel(**inputs):
    prec = DEFAULT_PREC
    from concourse.bass_utils import run_bass_kernel_spmd

    stage = os.environ.get("KERNEL_STAGE", "full")
    in_maps, flags = _prep_inputs(inputs, prec)
    key = (prec, stage, *flags)
    if key not in _BUILD_CACHE:
        _BUILD_CACHE[key] = _build(prec, *flags, stage=stage)
    nc = _BUILD_CACHE[key]

    res = run_bass_kernel_spmd(
        nc,
        in_maps,
        core_ids=list(range(NCORES)),
        trace=bool(int(os.environ.get("KERNEL_TRACE", "0"))),
    )
    def unperm(o):
        o = o.reshape(BPC, 8, 4, 2, 8, 8, C).transpose(0, 1, 4, 2, 3, 5, 6)
        return o.reshape(BPC, L, C)

    out = np.concatenate(
        [unperm(r["o"]) for r in res.results], axis=0
    ).astype(np.float32)
    if bool(int(os.environ.get("KERNEL_TRACE", "0"))):
        kernel.last_result = res
    return out


kernel.last_result = None


# revision 59
# speedup vs baseline: 1.0223x; 1.0223x over previous
"""Trainium2 Bass kernel for a Swin-style transformer block.

Reference computation (per image, H=W=64, C=384, 12 heads, 8x8 windows):
  x -> LN1 -> qkv -> windowed MHA (+rel-pos bias) -> proj -> +x
    -> LN2 -> fc1 -> ReLU6 -> fc2 -> +residual

Sharding: data-parallel over batch (16 images -> 8 cores x 2 images).

Per-core kernel design notes:
 - Tokens are processed window-major: tiles of 128 tokens = one "window pair"
   (two 8x8 windows); 4 window pairs = one 512-token chunk; 16 chunks/core.
 - LayerNorm gamma/beta are folded into the following matmul's weights and
   bias host-side (exact), so the normalized tile (x-mu)*rstd is transposed
   straight into matmul layout with SBUF-SBUF DMA transposes (no PE identity
   matmuls, no PSUM evacuation traffic for the transposes).
 - q/k are head-split with cheap HWDGE DMAs (SP + Act queues) into
   [32, 12, tok] tiles; V is computed full-width (both windows of the pair
   on 128 partitions) with a small shift DMA for half B.  All matmul
   operands and outputs sit at partition base 0 (quadrant tile_position
   matmuls are unreliable on this device stack).  Logits fill single-bank
   PSUM tiles 8 (half, head)-slots at a time so exp of bank b overlaps QK
   fills of bank b+1; AV/proj trail one window pair behind QK so the
   exp->exb chain latency hides behind PE work.  V is augmented with a
   ones column and attnT (exponentiated, bias-folded) is the stationary
   operand of attnT.T @ [V|1]; the output holds both the unnormalized
   attention output and the softmax denominator, normalized with one
   reciprocal + multiply.  No max-subtraction (logits are bounded for this
   distribution).  The DMA transposes back to token-major also lift half B
   to partitions 64-127 for free.
 - 5-stage staggered software pipeline (load+LN1 / qkv+attention+proj+LN2 /
   fp8 cast / fc1+clamp / fc2+store) so every DMA-transpose result is
   consumed a full round after issue (in-order engine queues head-of-line
   block otherwise).
 - The relative-position bias is folded in as a precomputed exp(bias)
   elementwise multiply on gpsimd (exp(l+b) = exp(l)*exp(b)).
 - MLP stays feature-major end to end; ReLU6 splits between DVE (fused
   clamp evac) and Act+gpsimd by feature-chunk parity.  KERNEL_PREC=
   bf16-mlp8 runs fc1/fc2 in fp8e4 DoubleRow (2x fewer PE cycles) but its
   ~3e-2 rel err exceeds the 2e-2 gate, so bf16 is the default.
"""

import os
import numpy as np

# ---------------------------------------------------------------- constants
B, L, C = 16, 4096, 384
HEADS, WS, HD = 12, 8, 32
MLP = 1536
NCORES = 8
BPC = B // NCORES          # images per core
T = BPC * L                # tokens per core
H = W = 64
EPS = 1e-5
NWIN = BPC * (H // WS) * (W // WS)   # 128 windows/core
NWP = NWIN // 2                      # 64 window pairs
WP_PER_CHUNK = int(os.environ.get("KERNEL_WP", "4"))   # window pairs per chunk
TOK = WP_PER_CHUNK * 128             # tokens per chunk
NCHUNK = NWP // WP_PER_CHUNK         # 16

DEFAULT_PREC = os.environ.get("KERNEL_PREC", "bf16")

_BUILD_CACHE = {}


def _rel_pos_index():
    coords = np.stack(np.meshgrid(np.arange(WS), np.arange(WS), indexing="ij"))
    cf = coords.reshape(2, -1)
    rel = cf[:, :, None] - cf[:, None, :]
    rel = rel.transpose(1, 2, 0).astype(np.int64)
    rel[:, :, 0] += WS - 1
    rel[:, :, 1] += WS - 1
    rel[:, :, 0] *= 2 * WS - 1
    return rel.sum(-1)  # (64, 64)


def _split_excess_waits(nc, max_waits=1):
    """TRN2 instructions encode a single semaphore-wait slot; Tile's exit
    drain (and occasionally other instructions) carries several.  Hoist the
    excess into standalone event-semaphore waits on the same engine."""
    import concourse.mybir as mybir

    uid = [0]
    for fn in nc.m.functions:
        for bb in fn.blocks:
            out = []
            for ins in bb.instructions:
                si = ins.sync_info
                if si is not None and si.on_wait and len(si.on_wait) > max_waits:
                    waits = list(si.on_wait)
                    excess, keep = waits[:-max_waits], waits[-max_waits:]
                    for w in excess:
                        uid[0] += 1
                        ev = mybir.InstEventSemaphore(
                            name=f"WSPLIT-{uid[0]}",
                            engine=ins.engine,
                            ins=[],
                            outs=[],
                            sync_info=mybir.SyncInfo(on_wait=[w], on_update=[]),
                        )
                        nc.register_instruction(ev, overwrite=True)
                        out.append(ev)
                    si.on_wait = keep
                out.append(ins)
            bb.instructions = out


def _build(prec, has_fc1b, has_projb, has_fc2b, has_vb=False, stage="full"):
    import concourse.bass as bass
    import concourse.mybir as mybir
    from concourse.tile import TileContext

    f32 = mybir.dt.float32
    bf16 = mybir.dt.bfloat16
    fp8 = mybir.dt.float8e4
    mlp8 = prec.endswith("-mlp8")
    base_prec = prec.replace("-mlp8", "")
    assert base_prec == "bf16", prec
    DT_D = DT_A = bf16
    DT_M = fp8 if mlp8 else DT_D          # MLP operand dtype

    nc = bass.Bass()

    x_d = nc.declare_dram_parameter("x", [NWP, 128, C], f32, isOutput=False)
    o_d = nc.declare_dram_parameter("o", [NWP, 128, C], f32, isOutput=True)
    wqkvT_d = nc.declare_dram_parameter("wqkvT", [C, 3 * C], DT_D, isOutput=False)
    wpT_d = nc.declare_dram_parameter("wpT", [C, C], DT_D, isOutput=False)
    w1T_d = nc.declare_dram_parameter(
        "w1T", [512 if mlp8 else C, MLP], DT_M, isOutput=False
    )
    w2T_d = nc.declare_dram_parameter("w2T", [MLP, C], DT_M, isOutput=False)
    qkb_d = nc.declare_dram_parameter("qkb", [C, 2], f32, isOutput=False)
    vb_d = nc.declare_dram_parameter("vb", [C], f32, isOutput=False)
    fc1b_d = nc.declare_dram_parameter("fc1b", [MLP], f32, isOutput=False)
    cb_d = nc.declare_dram_parameter("cb", [C, 2], f32, isOutput=False)  # proj_b, fc2_b
    expb_d = nc.declare_dram_parameter("expb", [64, 2 * HEADS, 64], DT_A, isOutput=False)

    AL = mybir.AluOpType
    AF = mybir.ActivationFunctionType
    DR = mybir.MatmulPerfMode.DoubleRow

    from contextlib import ExitStack

    with TileContext(nc) as tc, ExitStack() as _stk:
            pool = lambda name, bufs, **kw: _stk.enter_context(
                tc.tile_pool(name=name, bufs=bufs, **kw)
            )
            consts = pool("consts", 1)
            px = pool("px", int(os.environ.get("KB_X", "3")))
            pt = pool("pt", int(os.environ.get("KB_T", "2")))
            pstat = pool("pstat", int(os.environ.get("KB_STAT", "2")))
            pxlnT = pool("pxlnT", int(os.environ.get("KB_XLNT", "2")))
            pqkT = pool("pqkT", 2)
            pqh = pool("pqh", int(os.environ.get("KB_QH", "1")))
            pV = pool("pV", int(os.environ.get("KB_V", "1")))
            pexp = pool("pexp", int(os.environ.get("KB_EXP", "3")))
            po = pool("po", int(os.environ.get("KB_O", "2")))
            poT = pool("poT", 2)
            px2 = pool("px2", int(os.environ.get("KB_X2", "4")))
            ph2T = pool("ph2T", 2)
            ph3 = pool("ph3", int(os.environ.get("KB_H3", "2")))
            pout = pool("pout", 2)
            _pb = [int(v) for v in os.environ.get("KERNEL_PSUM", "1,3,2,2").split(",")]
            psMM = pool("psMM", _pb[0], space="PSUM")
            psQK = pool("psQK", _pb[1], space="PSUM")
            psAV = pool("psAV", _pb[2], space="PSUM")
            psF = pool("psF", _pb[3], space="PSUM")
            # ---------------- constants into SBUF
            wqkvT = consts.tile([128, 3, 3 * C], DT_D, tag="wqkvT")
            nc.scalar.dma_start(
                out=wqkvT, in_=wqkvT_d[:].rearrange("(a p) o -> p a o", p=128)
            )
            wpT = consts.tile([128, 3, C], DT_D, tag="wpT")
            nc.scalar.dma_start(out=wpT, in_=wpT_d[:].rearrange("(a p) o -> p a o", p=128))
            w1T = consts.tile([128, 4 if mlp8 else 3, MLP], DT_M, tag="w1T")
            nc.scalar.dma_start(out=w1T, in_=w1T_d[:].rearrange("(a p) o -> p a o", p=128))
            w2T = consts.tile([128, 12, C], DT_M, tag="w2T")
            nc.scalar.dma_start(out=w2T, in_=w2T_d[:].rearrange("(a p) o -> p a o", p=128))
            qkb = consts.tile([128, 3, 2], f32, tag="qkb")
            nc.sync.dma_start(out=qkb, in_=qkb_d[:].rearrange("(a p) s -> p a s", p=128))
            expb = consts.tile([64, 2 * HEADS, 64], DT_A, tag="expb")
            nc.sync.dma_start(out=expb, in_=expb_d[:])
            vb = consts.tile([128, C], f32, tag="vb")
            nc.gpsimd.dma_start(out=vb, in_=vb_d[:].partition_broadcast(128))
            epst = consts.tile([128, 1], f32, tag="eps")
            nc.vector.memset(epst[:], EPS)
            fc1b = None
            if has_fc1b:
                fc1b = consts.tile([128, 12], f32, tag="fc1b")
                nc.sync.dma_start(
                    out=fc1b, in_=fc1b_d[:].rearrange("(a p) -> p a", p=128)
                )
            cbias = None
            if has_projb or has_fc2b:
                cbias = consts.tile([128, C, 2], f32, tag="cb")
                nc.gpsimd.dma_start(
                    out=cbias, in_=cb_d[:].partition_broadcast(128)
                )
            zero8 = None
            if mlp8:
                zero8 = consts.tile([128, TOK], fp8, tag="zero8")
                nc.vector.memset(zero8[:], 0.0)

            tt_eng = os.environ.get("KERNEL_TT", "dve")

            def ln_stage_j(src_tile, dst_T_tiles, j):
                """token-major LN for one window pair: src [128,384] f32 ->
                dst_T [128, 128j:128j+128] bf16 (transposed; gamma/beta
                pre-folded into the next matmul)."""
                if True:
                    st = pstat.tile([128, 6], f32, tag=f"bn{j}")
                    nc.vector.bn_stats(out=st, in_=src_tile[:])
                    mv = pstat.tile([128, 2], f32, tag=f"mv{j}")
                    nc.vector.bn_aggr(out=mv, in_=st)
                    # rstd = exp(-0.5*ln(var+eps)): keeps all ACT funcs in the
                    # natural_log_exp table set (one table load for the kernel)
                    rst = pstat.tile([128, 2], f32, tag=f"rs{j}")
                    nc.scalar.activation(
                        out=rst[:, 0:1], in_=mv[:, 1:2], func=AF.Ln,
                        bias=epst[:, 0:1], scale=1.0,
                    )
                    nc.scalar.activation(
                        out=rst[:, 1:2], in_=rst[:, 0:1], func=AF.Exp, bias=0.0, scale=-0.5
                    )
                    tt = pt.tile([128, C], DT_D, tag=f"t{j}")
                    if tt_eng == "act":
                        nmr = pstat.tile([128, 1], f32, tag=f"nm{j}")
                        nc.vector.tensor_scalar(
                            out=nmr[:], in0=mv[:, 0:1], scalar1=rst[:, 1:2],
                            scalar2=-1.0, op0=AL.mult, op1=AL.mult,
                        )
                        nc.scalar.activation(
                            out=tt[:], in_=src_tile[:], func=AF.Identity,
                            bias=nmr[:, 0:1], scale=rst[:, 1:2],
                        )
                    else:
                        nc.vector.tensor_scalar(
                            out=tt[:],
                            in0=src_tile[:],
                            scalar1=mv[:, 0:1],
                            scalar2=rst[:, 1:2],
                            op0=AL.subtract,
                            op1=AL.mult,
                        )
                    for cc in range(3):
                        nc.sync.dma_start_transpose(
                            dst_T_tiles[cc][:, 128 * j : 128 * (j + 1)],
                            tt[:, 128 * cc : 128 * (cc + 1)],
                        )

            def ln_stage(src_tiles, dst_T_tiles):
                for j in range(WP_PER_CHUNK):
                    ln_stage_j(src_tiles[j], dst_T_tiles, j)

            # ====== 5-stage software pipeline, staggered so every
            # DMA-transpose result is consumed a full round after issue
            # (in-order engine queues head-of-line block otherwise):
            #   round r emits A(r), B(r-1), C(r-1), D(r-2), E(r-3)
            def stageA(ci):
                wp0 = ci * WP_PER_CHUNK
                x_tm = []
                for j in range(WP_PER_CHUNK):
                    xt = px.tile([128, C], f32, tag=f"x{j}")
                    nc.sync.dma_start(out=xt[:], in_=x_d[wp0 + j])
                    x_tm.append(xt)
                xlnT = [pxlnT.tile([128, TOK], DT_D, tag=f"xlnT{cc}", name=f"xlnT{cc}") for cc in range(3)]
                ln_stage(x_tm, xlnT)
                return {"wp0": wp0, "x_tm": x_tm, "xlnT": xlnT}

            def passthrough(st):
                for tt in range(WP_PER_CHUNK):
                    out_t = pout.tile([128, C], f32, tag=f"out{tt}")
                    nc.vector.tensor_copy(out=out_t[:], in_=st["x_tm"][tt][:])
                    nc.sync.dma_start(out=o_d[st["wp0"] + tt], in_=out_t[:])

            def stageB(st):
                x_tm, xlnT = st["x_tm"], st["xlnT"]
                # ---- qkv -> qT/kT [128,512] per output chunk (4 heads
                # each), then head-split into [32, 12, 512] via strided
                # HWDGE DMAs (quadrant matmuls are unreliable on device;
                # operands must sit at partition base 0)
                qT, kT = [], []
                for oc in range(3):
                    for which, dst_list, bcol in (("q", qT, 0), ("k", kT, 1)):
                        ps = psMM.tile([128, 512], f32, tag="mm")
                        for kc in range(3):
                            col0 = (0 if which == "q" else C) + 128 * oc
                            nc.tensor.matmul(
                                ps[:, 0:TOK],
                                lhsT=wqkvT[:, kc, col0 : col0 + 128],
                                rhs=xlnT[kc][:],
                                start=(kc == 0),
                                stop=(kc == 2),
                            )
                        dst = pqkT.tile([128, TOK], DT_A, tag=f"{which}T{oc}")
                        # whole-tile evac (the head-split gather DMA below
                        # reads a partition-rearranged view; split writes
                        # break its dependency tracking): q on Act, k on DVE
                        if which == "q":
                            nc.scalar.activation(
                                out=dst[:], in_=ps[:, 0:TOK], func=AF.Identity,
                                bias=qkb[:, oc, bcol : bcol + 1], scale=1.0,
                            )
                        else:
                            nc.vector.tensor_scalar(
                                out=dst[:], in0=ps[:, 0:TOK],
                                scalar1=qkb[:, oc, bcol : bcol + 1], scalar2=None,
                                op0=AL.add,
                            )
                        dst_list.append(dst)
                qh = pqh.tile([32, HEADS, TOK], DT_A, tag="qh", name="qh")
                kh = pqh.tile([32, HEADS, TOK], DT_A, tag="kh", name="kh")
                # plain partition-slice reads (a partition-rearranged view
                # in one DMA defeats the dependency tracker)
                for oc in range(3):
                    for hh in range(4):
                        nc.sync.dma_start(
                            out=qh[:, 4 * oc + hh, :],
                            in_=qT[oc][32 * hh : 32 * hh + 32, :],
                        )
                        nc.scalar.dma_start(
                            out=kh[:, 4 * oc + hh, :],
                            in_=kT[oc][32 * hh : 32 * hh + 32, :],
                        )

                # ---- V (full window pair on 128 partitions, ones column);
                # half B shifted to partition base 0 with a small DMA
                V_aug, V_augB = [], []
                for j in range(WP_PER_CHUNK):
                    ps = psMM.tile([128, 512], f32, tag="mm")
                    for kc in range(3):
                        nc.tensor.matmul(
                            ps[:, :C],
                            lhsT=xlnT[kc][:, 128 * j : 128 * (j + 1)],
                            rhs=wqkvT[:, kc, 2 * C : 3 * C],
                            start=(kc == 0),
                            stop=(kc == 2),
                        )
                    va = pV.tile([128, HEADS, HD + 1], DT_A, tag=f"va{j}", name=f"va{j}")
                    if not has_vb and os.environ.get("KERNEL_VAE", "dve") == "act":
                        nc.scalar.activation(
                            out=va[:, 0:6, 0:HD],
                            in_=ps[:, 0 : C // 2].rearrange("p (h d) -> p h d", h=HEADS // 2),
                            func=AF.Identity, bias=0.0, scale=1.0,
                        )
                        nc.vector.tensor_scalar(
                            out=va[:, 6:12, 0:HD],
                            in0=ps[:, C // 2 : C].rearrange("p (h d) -> p h d", h=HEADS // 2),
                            scalar1=0.0, scalar2=None, op0=AL.add,
                        )
                    else:
                        nc.vector.scalar_tensor_tensor(
                            out=va[:, :, 0:HD],
                            in0=ps[:, :C].rearrange("p (h d) -> p h d", h=HEADS),
                            scalar=0.0,
                            in1=vb[:].rearrange("p (h d) -> p h d", h=HEADS),
                            op0=AL.add,
                            op1=AL.add,
                        )
                    nc.vector.memset(va[:, :, HD : HD + 1], 1.0)
                    vab = pV.tile([64, HEADS, HD + 1], DT_A, tag=f"vab{j}", name=f"vab{j}")
                    nc.sync.dma_start(out=vab[:], in_=va[64:128])
                    V_aug.append(va)
                    V_augB.append(vab)

                if stage == "qkv":
                    passthrough(st)
                    return False

                # ---- attention; AV/proj run one window pair behind QK so
                # the exp->exb chain latency is hidden behind PE work
                oT = [poT.tile([128, TOK], DT_A, tag=f"oT{cc}", name=f"oT{cc}") for cc in range(3)]
                x2_tm = [None] * WP_PER_CHUNK
                exb_eng = os.environ.get("KERNEL_EXB", "pool")

                def consume(j, exb):
                    for half in (0, 1):
                        vsrc = V_aug[j][0:64] if half == 0 else V_augB[j]
                        psav = psAV.tile([64, HEADS, HD + 2], f32, tag="av", name="psav")
                        for h in range(HEADS):
                            nc.tensor.matmul(
                                psav[:, h, 0 : HD + 1],
                                lhsT=exb[:, HEADS * half + h, :],
                                rhs=vsrc[:, h, 0 : HD + 1],
                                start=True,
                                stop=True,
                                skip_group_check=True,
                            )
                        rec = pstat.tile([64, HEADS], f32, tag="rec")
                        nc.vector.reciprocal(out=rec[:], in_=psav[:, :, HD : HD + 1])
                        ow = po.tile(
                            [64, C], DT_A, tag=f"o{2 * j + half}", name=f"o{2 * j + half}"
                        )
                        nc.vector.tensor_tensor(
                            out=ow[:].rearrange("p (h d) -> p h d", h=HEADS),
                            in0=psav[:, :, 0:HD],
                            in1=rec[:, :, None].broadcast_to([64, HEADS, HD]),
                            op=AL.mult,
                        )
                        # transpose (also lifts half B to token partitions
                        # 64-127)
                        for cc in range(3):
                            nc.sync.dma_start_transpose(
                                oT[cc][:, 128 * j + 64 * half : 128 * j + 64 * half + 64],
                                ow[:, 128 * cc : 128 * (cc + 1)],
                            )

                h2T = [ph2T.tile([128, TOK], DT_D, tag=f"h2T{cc}", name=f"h2T{cc}") for cc in range(3)]

                def proj(tt):
                    ps = psMM.tile([128, 512], f32, tag="mm")
                    for cc in range(3):
                        nc.tensor.matmul(
                            ps[:, :C],
                            lhsT=oT[cc][:, 128 * tt : 128 * (tt + 1)],
                            rhs=wpT[:, cc, :],
                            start=(cc == 0),
                            stop=(cc == 2),
                        )
                    x2 = px2.tile([128, C], f32, tag=f"x2_{tt}")
                    nc.vector.scalar_tensor_tensor(
                        out=x2[:], in0=ps[:, :C], scalar=0.0, in1=x_tm[tt][:],
                        op0=AL.add, op1=AL.add,
                    )
                    if has_projb:
                        nc.vector.tensor_add(x2[:], x2[:], cbias[:, :, 0])
                    x2_tm[tt] = x2
                    # LN2 for this window pair immediately: its transposes
                    # issue a full round before fc1 consumes h2T
                    ln_stage_j(x2, h2T, tt)

                pend_av = []
                for j in range(WP_PER_CHUNK):
                    ja = 128 * j
                    # 24 (half, head) logit slots, 8 per single-bank PSUM
                    # tile so exp of bank b overlaps QK fills of bank b+1
                    psq = [
                        psQK.tile([64, 8, 64], f32, tag="qk", name="psq")
                        for _ in range(3)
                    ]
                    ex = pexp.tile([64, 2 * HEADS, 64], DT_A, tag="ex")
                    exb = pexp.tile([64, 2 * HEADS, 64], DT_A, tag="exb")
                    for b in range(3):
                        for s in range(8):
                            slot = 8 * b + s
                            half, h = slot // HEADS, slot % HEADS
                            t0 = ja + 64 * half
                            nc.tensor.matmul(
                                psq[b][:, s, :],
                                lhsT=kh[:, h, t0 : t0 + 64],
                                rhs=qh[:, h, t0 : t0 + 64],
                                start=True,
                                stop=True,
                                skip_group_check=True,
                            )
                        if stage == "qkm":
                            continue
                        nc.scalar.activation(
                            out=ex[:, 8 * b : 8 * b + 8, :],
                            in_=psq[b][:],
                            func=AF.Exp,
                        )
                        if exb_eng == "pool":
                            nc.gpsimd.tensor_tensor(
                                out=exb[:, 8 * b : 8 * b + 8, :],
                                in0=ex[:, 8 * b : 8 * b + 8, :],
                                in1=expb[:, 8 * b : 8 * b + 8, :],
                                op=AL.mult,
                            )
                        else:
                            nc.vector.tensor_mul(
                                exb[:, 8 * b : 8 * b + 8, :],
                                ex[:, 8 * b : 8 * b + 8, :],
                                expb[:, 8 * b : 8 * b + 8, :],
                            )
                    if stage in ("qk", "qkm"):
                        continue
                    pend_av.append((j, exb))
                    if len(pend_av) > 1:
                        jc, e = pend_av.pop(0)
                        consume(jc, e)
                        if jc > 0:
                            proj(jc - 1)
                if stage in ("att", "qk", "qkm"):
                    while pend_av:
                        consume(*pend_av.pop(0))
                    passthrough(st)
                    return False
                while pend_av:
                    jc, e = pend_av.pop(0)
                    consume(jc, e)
                    if jc > 0:
                        proj(jc - 1)
                proj(WP_PER_CHUNK - 1)
                st["x2_tm"] = x2_tm
                st["h2T"] = h2T
                return True

            def stageC(st):
                h2T = st["h2T"]
                if mlp8:
                    # single [128, 4, 512] fp8 tile: 3 cast chunks + 1 zero
                    # chunk so fc1 runs as two clean DoubleRow matmuls
                    h2T8 = ph2T.tile([128, 4, TOK], fp8, tag="h2T8", name="h2T8")
                    for cc in range(3):
                        nc.gpsimd.dma_start(out=h2T8[:, cc, :], in_=h2T[cc][:])
                    nc.gpsimd.dma_start(out=h2T8[:, 3, :], in_=zero8[:])
                    st["h2T8"] = h2T8
                st["h2T"] = h2T

            def stageD(st):
                # ---- fc1 + relu6 (feature-major); h3 stored as 6
                # pair-tiles [128, 2, 512] so fc2 can consume DoubleRow
                # pairs directly
                h2T = st["h2T"]
                h3 = [
                    ph3.tile([128, 2, TOK], DT_M, tag=f"h3_{p}", name=f"h3_{p}")
                    for p in range(6)
                ]
                for mc in range(12):
                    ps = psF.tile([128, 512], f32, tag="mm2")
                    if mlp8:
                        h2T8 = st["h2T8"]
                        for p in range(2):
                            nc.tensor.matmul(
                                ps[:, 0:TOK],
                                lhsT=w1T[:, 2 * p : 2 * p + 2, 128 * mc : 128 * (mc + 1)],
                                rhs=h2T8[:, 2 * p : 2 * p + 2, :],
                                start=(p == 0), stop=(p == 1),
                                perf_mode=DR,
                            )
                    else:
                        for kc in range(3):
                            nc.tensor.matmul(
                                ps[:, 0:TOK],
                                lhsT=w1T[:, kc, 128 * mc : 128 * (mc + 1)],
                                rhs=h2T[kc][:],
                                start=(kc == 0),
                                stop=(kc == 2),
                            )
                    h3t = h3[mc // 2][:, mc % 2, :]
                    h3_mode = os.environ.get("KERNEL_H3", "split")
                    if h3_mode == "split":
                        h3_mode = "actpool" if mc % 2 else "dve"
                    if h3_mode == "actpool":
                        # relu evac on Act (fc1 bias is per-partition here:
                        # features on partitions), min(6) on gpsimd
                        nc.scalar.activation(
                            out=h3t, in_=ps[:, 0:TOK], func=AF.Relu,
                            bias=(fc1b[:, mc : mc + 1] if has_fc1b else 0.0),
                            scale=1.0,
                        )
                        nc.gpsimd.tensor_scalar(
                            out=h3t, in0=h3t, scalar1=6.0, scalar2=None,
                            op0=AL.min,
                        )
                    elif has_fc1b:
                        nc.vector.tensor_scalar(
                            out=h3t, in0=ps[:, 0:TOK],
                            scalar1=fc1b[:, mc : mc + 1], scalar2=0.0,
                            op0=AL.add, op1=AL.max,
                        )
                        nc.gpsimd.tensor_scalar(
                            out=h3t, in0=h3t, scalar1=6.0, scalar2=None,
                            op0=AL.min,
                        )
                    else:
                        nc.vector.tensor_scalar(
                            out=h3t, in0=ps[:, 0:TOK], scalar1=0.0, scalar2=6.0,
                            op0=AL.max, op1=AL.min,
                        )
                st["h3"] = h3

            def stageE(st):
                wp0, x2_tm, h3 = st["wp0"], st["x2_tm"], st["h3"]
                for tt in range(WP_PER_CHUNK):
                    ps = psF.tile([128, 512], f32, tag="mm2")
                    if mlp8:
                        for p in range(6):
                            nc.tensor.matmul(
                                ps[:, :C],
                                lhsT=h3[p][:, :, 128 * tt : 128 * (tt + 1)],
                                rhs=w2T[:, 2 * p : 2 * p + 2, :],
                                start=(p == 0), stop=(p == 5),
                                perf_mode=DR,
                            )
                    else:
                        for p in range(6):
                            for i in range(2):
                                mc = 2 * p + i
                                nc.tensor.matmul(
                                    ps[:, :C],
                                    lhsT=h3[p][:, i, 128 * tt : 128 * (tt + 1)],
                                    rhs=w2T[:, mc, :],
                                    start=(mc == 0),
                                    stop=(mc == 11),
                                )
                    out_t = pout.tile([128, C], f32, tag=f"out{tt}")
                    nc.vector.scalar_tensor_tensor(
                        out=out_t[:], in0=ps[:, :C], scalar=0.0, in1=x2_tm[tt][:],
                        op0=AL.add, op1=AL.add,
                    )
                    if has_fc2b:
                        nc.vector.tensor_add(out_t[:], out_t[:], cbias[:, :, 1])
                    nc.sync.dma_start(out=o_d[wp0 + tt], in_=out_t[:])

            sts = {}
            for r in range(NCHUNK + 3):
                if r < NCHUNK:
                    sts[r] = stageA(r)
                    if stage == "ln":
                        passthrough(sts[r])
                        sts.pop(r)
                        continue
                if stage == "ln":
                    continue
                if r - 1 in sts and "x2_tm" not in sts[r - 1]:
                    if not stageB(sts[r - 1]):
                        sts.pop(r - 1)
                        continue
                    stageC(sts[r - 1])
                if r - 2 in sts:
                    stageD(sts[r - 2])
                if r - 3 in sts:
                    stageE(sts[r - 3])
                    sts.pop(r - 3)

    _split_excess_waits(nc, 1)
    return nc


def _prep_inputs(inputs, prec):
    import ml_dtypes

    bf16 = ml_dtypes.bfloat16
    fp8 = ml_dtypes.float8_e4m3
    mlp8 = prec.endswith("-mlp8")
    dt_d = bf16
    dt_a = bf16
    dt_m = fp8 if mlp8 else dt_d

    f = lambda a: np.ascontiguousarray(np.asarray(a, dtype=np.float32))
    x = f(inputs["x"])
    qkv_w, qkv_b = f(inputs["qkv_w"]), f(inputs["qkv_b"])
    ln1_g, ln1_b = f(inputs["ln1_g"]), f(inputs["ln1_b"])
    ln2_g, ln2_b = f(inputs["ln2_g"]), f(inputs["ln2_b"])
    # fold LN1 gamma/beta into qkv (exact):  W @ (g*z + b) = (W*g) @ z + W @ b
    qkv_w_f = qkv_w * ln1_g[None, :]
    qkv_b_f = qkv_b + qkv_w @ ln1_b
    scale = 1.0 / np.sqrt(HD)
    wq = qkv_w_f[0:C] * scale
    wqkvT = np.concatenate([wq.T, qkv_w_f[C : 2 * C].T, qkv_w_f[2 * C :].T], axis=1)
    qkb = np.stack([qkv_b_f[0:C] * scale, qkv_b_f[C : 2 * C]], axis=1)
    vb = qkv_b_f[2 * C :]
    wpT = f(inputs["proj_w"]).T
    # fold LN2 gamma/beta into fc1 (exact)
    fc1_w, fc1_b = f(inputs["fc1_w"]), f(inputs["fc1_b"])
    fc1_w_f = fc1_w * ln2_g[None, :]
    fc1_b_f = fc1_b + fc1_w @ ln2_b
    w1T = fc1_w_f.T
    if mlp8:
        w1T = np.concatenate([w1T, np.zeros((512 - C, MLP), np.float32)], axis=0)
    w2T = f(inputs["fc2_w"]).T
    cb = np.stack([f(inputs["proj_b"]), f(inputs["fc2_b"])], axis=1)

    rel = _rel_pos_index()
    bias = f(inputs["rpb_table"])[rel]          # [n, m, HEADS]
    expb1 = np.exp(bias.transpose(1, 2, 0))     # [m, HEADS, n]
    expb = np.concatenate([expb1, expb1], axis=1)  # [m, 2*HEADS, n] half-major

    common = {
        "wqkvT": np.ascontiguousarray(wqkvT.astype(dt_d)),
        "wpT": np.ascontiguousarray(wpT.astype(dt_d)),
        "w1T": np.ascontiguousarray(w1T.astype(dt_m)),
        "w2T": np.ascontiguousarray(w2T.astype(dt_m)),
        "qkb": np.ascontiguousarray(qkb),
        "vb": np.ascontiguousarray(vb),
        "fc1b": np.ascontiguousarray(fc1_b_f),
        "cb": np.ascontiguousarray(cb),
        "expb": np.ascontiguousarray(expb.astype(dt_a)),
    }
    flags = (
        bool(np.any(fc1_b_f)),
        bool(np.any(cb[:, 0])),
        bool(np.any(cb[:, 1])),
        bool(np.any(vb)),
    )
    in_maps = []
    for c in range(NCORES):
        m = dict(common)
        xc = x[c * BPC : (c + 1) * BPC].reshape(BPC, 8, 8, 4, 2, 8, C)
        m["x"] = np.ascontiguousarray(
            xc.transpose(0, 1, 3, 4, 2, 5, 6).reshape(NWP, 128, C)
        )
        in_maps.append(m)
    return in_maps, flags


def kernel(**inputs):
    prec = DEFAULT_PREC
    from concourse.bass_utils import run_bass_kernel_spmd

    stage = os.environ.get("KERNEL_STAGE", "full")
    in_maps, flags = _prep_inputs(inputs, prec)
    key = (prec, stage, *flags)
    if key not in _BUILD_CACHE:
        _BUILD_CACHE[key] = _build(prec, *flags, stage=stage)
    nc = _BUILD_CACHE[key]

    res = run_bass_kernel_spmd(
        nc,
        in_maps,
        core_ids=list(range(NCORES)),
        trace=bool(int(os.environ.get("KERNEL_TRACE", "0"))),
    )
    def unperm(o):
        o = o.reshape(BPC, 8, 4, 2, 8, 8, C).transpose(0, 1, 4, 2, 3, 5, 6)
        return o.reshape(BPC, L, C)

    out = np.concatenate(
        [unperm(r["o"]) for r in res.results], axis=0
    ).astype(np.float32)
    if bool(int(os.environ.get("KERNEL_TRACE", "0"))):
        kernel.last_result = res
    return out


kernel.last_result = None
